# revision 9
# baseline (speedup 1.0000x reference)
"""Trainium2 Bass kernel for stereo disparity correlation (cost volume).

corr[b, d, h, w] = (1/C) * sum_c L[b,c,h,w] * R[b,c,h,w-d],  zero-padded w-d<0
x = concat([L, R], axis=1): [4, 64, 256, 512] f32, C=32, D=64.

Sharding: 8 cores = 4 batches x 2 H-halves. Each core handles
x_shard [64, 128, 512] -> out_shard [64(D), 128(H), 512(W)].

Algorithm per core (memory-bound; ~64MB HBM traffic/core):
  1. Load L,R rows (4 h per DMA, [128p, 512] f32), cast to 16-bit on DVE
     (L scaled by 1/C; R into a 63-col zero-padded tile).
  2. Per (h, wblock k of 128): PE Gram G[r, n] = sum_c Ls[c, 128k+r] *
     Rpad[c, 128k+n], K=32, M=128, N=191.  out[w=128k+r, d] = G[r, r+63-d].
  3. Evict G (f32 PSUM -> 16-bit SBUF, ACT/DVE alternating), DMA 2h-batched
     to a flat DRAM scratch tile.
  4. Re-read the diagonal band via flat AP: off = r*(2*4*191+1) + h*764 +
     k*191 + d', d' = 63-d  -> band[r, k, d'] in SBUF (scalar HWDGE queue).
  5. PE transpose band k-slices -> PSUM [d', w], DVE-evict f32.
  6. DMA to output with negative outer step on d' (flips to d).
"""

import sys
from contextlib import ExitStack

import numpy as np

for _p in ("/opt/trn_rl_repo",):
    if _p not in sys.path:
        sys.path.insert(0, _p)

import concourse.bass as bass
import concourse.bacc as bacc
import concourse.tile as tile
from concourse import mybir
from concourse.bass_utils import run_bass_kernel_spmd

# problem constants (hardcoded per contract)
B, C2, H, W = 4, 64, 256, 512
C = 32
D = 64
N_CORES = 8
HS = H // 2          # h rows per core = 128
KB = W // 128        # w-blocks per row = 4
NCOL = 191           # Gram columns per block (n = r + d', r<=127, d'<=63)
GH = 4               # h rows loaded per input DMA group

FP32 = mybir.dt.float32
GDT = mybir.dt.bfloat16     # 16-bit compute/scratch dtype
NPDT = np.float32


def build_kernel(hs=HS, sim_safe=False):
    """sim_safe=True avoids split-partition DMA dest APs (slower, but the
    CoreSim shadow tracker handles them; HW handles both correctly)."""
    nc = bacc.Bacc(
        "TRN2", target_bir_lowering=False, debug=False, num_devices=N_CORES
    )
    x = nc.dram_tensor("x", [C2, hs, W], FP32, kind="ExternalInput").ap()
    ident = nc.dram_tensor("ident", [128, 128], GDT, kind="ExternalInput").ap()
    out = nc.dram_tensor("out", [D, hs, W], FP32, kind="ExternalOutput").ap()

    RSTRIDE = 2 * KB * NCOL  # per-r stride in the 2h scratch tile (=1528)

    with tile.TileContext(nc) as tc, ExitStack() as ctx:
        lio = ctx.enter_context(tc.tile_pool(name="lio", bufs=3))
        lcast = ctx.enter_context(tc.tile_pool(name="lcast", bufs=3))
        gbuf = ctx.enter_context(tc.tile_pool(name="gbuf", bufs=3))
        bbuf = ctx.enter_context(tc.tile_pool(name="bbuf", bufs=4))
        obuf = ctx.enter_context(tc.tile_pool(name="obuf", bufs=3))
        singles = ctx.enter_context(tc.tile_pool(name="singles", bufs=1))
        psg = ctx.enter_context(tc.tile_pool(name="psg", bufs=6, space="PSUM"))
        pso = ctx.enter_context(tc.tile_pool(name="pso", bufs=2, space="PSUM"))
        dram = ctx.enter_context(tc.tile_pool(name="dram", bufs=4, space="DRAM"))

        ident_t = singles.tile([128, 128], GDT)
        nc.sync.dma_start(out=ident_t[:], in_=ident)

        n_groups = hs // GH
        for g in range(n_groups):
            h0 = g * GH
            # ---- load 4 h rows of L and R, f32, partitions (h, c) ----
            lf32 = lio.tile([128, W], FP32, tag="lf32")
            rf32 = lio.tile([128, W], FP32, tag="rf32")
            if sim_safe:
                for hi in range(GH):
                    nc.sync.dma_start(
                        out=lf32[C * hi : C * hi + C, :], in_=x[0:C, h0 + hi, :]
                    )
                    nc.sync.dma_start(
                        out=rf32[C * hi : C * hi + C, :],
                        in_=x[C : 2 * C, h0 + hi, :],
                    )
            else:
                # batched: dest partition dim split (h, c) — lowers to
                # correct descriptors; one DMA per tensor per 4h
                nc.sync.dma_start(
                    out=lf32[:].rearrange("(h c) w -> h c w", h=GH),
                    in_=x[0:C, h0 : h0 + GH, :].transpose([1, 0, 2]),
                )
                nc.sync.dma_start(
                    out=rf32[:].rearrange("(h c) w -> h c w", h=GH),
                    in_=x[C : 2 * C, h0 : h0 + GH, :].transpose([1, 0, 2]),
                )
            # ---- cast to 16-bit (L scaled by 1/C), R zero-padded by 63 ----
            ls = lcast.tile([128, W], GDT, tag="ls")
            rpad = lcast.tile([128, 63 + W], GDT, tag="rpad")
            nc.vector.tensor_scalar_mul(ls[:], lf32[:], 1.0 / C)
            nc.vector.memset(rpad[:, 0:63], 0.0)
            nc.vector.tensor_copy(rpad[:, 63 : 63 + W], rf32[:])

            for hpair in range(GH // 2):
                # two h rows share gt/gd/psum_o/out tiles and their DMAs
                pso_t = pso.tile([128, W], GDT, tag="pso")
                gt = gbuf.tile([128, 2, KB, NCOL], GDT, tag="gt")
                for hi2 in range(2):
                    hi = hpair * 2 + hi2
                    prow = slice(C * hi, C * hi + C)
                    for k in range(KB):
                        psg_t = psg.tile([128, NCOL], FP32, tag="psg")
                        nc.tensor.matmul(
                            psg_t[:],
                            ls[prow, 128 * k : 128 * k + 128],
                            rpad[prow, 128 * k : 128 * k + NCOL],
                            start=True,
                            stop=True,
                            tile_position=(C * hi, 0),
                        )
                        # split evictions between ACT and DVE
                        if k % 2 == 0:
                            nc.scalar.copy(gt[:, hi2, k, :], psg_t[:])
                        else:
                            nc.vector.tensor_copy(gt[:, hi2, k, :], psg_t[:])
                # scratch roundtrip: write G (2h), read back diagonal bands
                gd = dram.tile([128, 2, KB, NCOL], GDT, tag="gd")
                nc.sync.dma_start(out=gd[:], in_=gt[:])
                gd_ap = gd[:]
                bands = []
                for hi2 in range(2):
                    band = bbuf.tile([128, KB, D], GDT, tag="band")
                    band_src = bass.AP(
                        tensor=gd_ap.tensor,
                        offset=gd_ap.offset + hi2 * KB * NCOL,
                        ap=[[RSTRIDE + 1, 128], [NCOL, KB], [1, D]],
                    )
                    # band reads go on the scalar HWDGE queue
                    nc.scalar.dma_start(out=band[:], in_=band_src)
                    bands.append(band)
                for hi2 in range(2):
                    for k in range(KB):
                        nc.tensor.transpose(
                            pso_t[64 * hi2 : 64 * hi2 + 64, 128 * k : 128 * k + 128],
                            bands[hi2][:, k, :],
                            ident_t[:],
                        )
                # evict both rows, write out with d flip
                out_t = obuf.tile([128, W], FP32, tag="out_t")
                nc.vector.tensor_copy(out_t[:], pso_t[:])
                ho = h0 + hpair * 2
                dst = bass.AP(
                    tensor=out.tensor,
                    offset=(D - 1) * hs * W + ho * W,
                    ap=[[W, 2], [-hs * W, D], [1, W]],
                )
                nc.sync.dma_start(out=dst, in_=out_t[:])

    nc.compile()
    return nc


_NC_CACHE = {}


def _get_nc(hs=HS, sim_safe=False):
    key = (hs, sim_safe)
    if key not in _NC_CACHE:
        _NC_CACHE[key] = build_kernel(hs, sim_safe)
    return _NC_CACHE[key]


def make_in_maps(x_full):
    ident = np.eye(128, dtype=mybir.dt.np(GDT))
    in_maps = []
    for core in range(N_CORES):
        b, hh = core // 2, core % 2
        shard = np.ascontiguousarray(x_full[b, :, hh * HS : (hh + 1) * HS, :])
        in_maps.append({"x": shard, "ident": ident})
    return in_maps


def assemble(results):
    out = np.empty((B, D, H, W), dtype=np.float32)
    for core in range(N_CORES):
        b, hh = core // 2, core % 2
        out[b, :, hh * HS : (hh + 1) * HS, :] = results[core]["out"]
    return out


def kernel(x, max_disp):
    import os

    x = np.asarray(x, dtype=np.float32)
    assert x.shape == (B, C2, H, W) and int(max_disp) == D
    nc = _get_nc(sim_safe=bool(int(os.environ.get("KSAFE", "0"))))
    res = run_bass_kernel_spmd(nc, make_in_maps(x), list(range(N_CORES)))
    return assemble(res.results)
